# revision 42
# baseline (speedup 1.0000x reference)
"""Trainium2 Bass kernel for nn_CamMemory (soft cross-entropy vs. memory bank).

Computes: x = normalize(inputs); logits = x @ features.T / TEMP;
loss = mean_b( lse(logits_b) - dot(softmax(targets_b), logits_b) )

Sharding: features/targets split row-wise (N dim) across 8 cores; inputs
replicated.  Each core returns partial stats (s, u, p) per batch row:
  s = sum_n exp(logits - SHIFT)      (partial sum-exp, fixed shift; |logits|<=21)
  u = sum_n exp(targets - 1)         (partial softmax denominator; targets in [0,1))
  p = sum_n exp(targets - 1)*logits  (partial weighted logit sum)
Host combines: loss = mean_b( SHIFT + log(sum s) - (sum p)/(sum u) ).

Layout strategy: the host stages each core's feature shard PRE-TRANSPOSED
and tiled (d on partitions) so the kernel needs no on-chip transposes at
all -- DRAM layout per chunk c of 256 bank rows is [128(d%128), 16(d/128),
256(n)], one contiguous 2MB block.  Inputs are likewise staged twice:
natural [64, 2048] (for the norm stats) and transposed [128, 16*64] (the
matmul stationary).  The kernel is then a pure stream:
  - SWDGE cast-DMA f32 DRAM -> bf16 SBUF, 2MB per n-chunk (8 chunks, all
    emitted upfront on the single SWDGE ring with one pool buf each so the
    ring never stalls; the last chunk is split into two contiguous-staged
    k-halves so the final matmul dependency lands 1MB early).
  - Per chunk: 16 accumulating bf16 matmuls (xT_k stationary, 256-col
    moving) into a PSUM tile [64, 256] = raw logits (no 1/TEMP, no norm).
  - ||x||^2 comes from the bf16 stationary itself (ACT square + a
    partition-reducing matmul chain against a ones column), so no
    natural-layout inputs load is needed at all.
  - Epilogue folds normalization in: exp(l_raw*inv - SHIFT) on ACT with
    per-partition scale inv = 1/(TEMP*||x_b||) and accum_out -> s.
    p uses inv-prescaled exp-targets: DVE mul + row-sum.
The DMA (16.8MB feature read at ~430GB/s) is the critical path; PE/ACT/
DVE all chase it with large headroom.
"""

import numpy as np

import concourse.bacc as bacc
import concourse.mybir as mybir
import concourse.tile as tile

B = 64
D = 2048
N = 16384
NUM_CORES = 8
NSH = N // NUM_CORES  # 2048 rows of features per core
TEMP = 0.05
SHIFT = 21.0  # |logits| <= (1/TEMP)*|x.f| <= 20*(1+eps) since both unit-norm

KC = D // 128     # 16 contraction chunks (d on partitions)
NCH = NSH // 256  # 8 n-chunks of 256 bank rows
CW = 256          # chunk width (bank rows per chunk)

F32 = mybir.dt.float32
BF16 = mybir.dt.bfloat16


def build_nc(debug=False):
    """Build the single-core Bass program (SPMD: same program, 8 shards)."""
    nc = bacc.Bacc("TRN2", target_bir_lowering=False, debug=debug)

    # inputs_t / features are host-staged in transposed+tiled layout, see _run.
    inputs_t_d = nc.dram_tensor("inputs_t", [128, KC * B], F32, kind="ExternalInput")
    targets_d = nc.dram_tensor("targets", [B, NSH], F32, kind="ExternalInput")
    features_d = nc.dram_tensor("features", [(NCH - 1) * 128, KC * CW], F32,
                                kind="ExternalInput")
    # last chunk staged half-major so each k-half is a fully contiguous
    # [128 x 8KB] region (strided or small sub-chunk reads drain slowly)
    features_last_d = nc.dram_tensor("features_last", [2 * 128, KC * CW // 2],
                                     F32, kind="ExternalInput")
    out_d = nc.dram_tensor("out", [B, 3], F32, kind="ExternalOutput")

    with tile.TileContext(nc) as tc:
        with (
            tc.tile_pool(name="small", bufs=1) as small,
            tc.tile_pool(name="ft", bufs=8) as ftp,
            tc.tile_pool(name="epi", bufs=4) as epi,
            tc.tile_pool(name="psw", bufs=1, space="PSUM") as psw,
            tc.tile_pool(name="psum", bufs=4, space="PSUM") as psp,
        ):
            # constants
            bias_m1 = small.tile([B, 1], F32)
            nc.vector.memset(bias_m1[:], -1.0)
            bias_shift = small.tile([B, 1], F32)
            nc.vector.memset(bias_shift[:], -float(SHIFT))
            wsrc = small.tile([128, CW], BF16)
            nc.vector.memset(wsrc[:], 0.0)
            ones_bf = small.tile([128, 1], BF16)
            nc.vector.memset(ones_bf[:], 1.0)

            def emit_ft_dma(c):
                ft = ftp.tile([128, KC * CW], BF16, name="ftc")
                if c < NCH - 1:
                    nc.gpsimd.dma_start(ft[:], features_d[c * 128:(c + 1) * 128, :])
                else:
                    # last chunk split along k into halves so the final
                    # matmul dependency (k=8..15) lands 1MB earlier; both
                    # DMA sides stay per-partition contiguous (smaller
                    # sub-chunks trigger an SDMA-15 slowdown)
                    q = KC * CW // 2
                    for h in range(2):
                        nc.gpsimd.dma_start(
                            ft[:, h * q:(h + 1) * q],
                            features_last_d[h * 128:(h + 1) * 128, :])
                return ft

            # chunk 0 rides the HWDGE ring as plain f32 (HWDGE's first byte
            # lands ~2us before SWDGE's) and is cast to bf16 on DVE; the
            # dual-ring overlap only lasts the first ~3us so it cannot
            # starve the SWDGE ring the way a full interleave does
            ft0 = ftp.tile([128, KC * CW], BF16, name="ftc")
            stg0 = small.tile([128, KC * CW], F32)
            nc.sync.dma_start(stg0[:], features_d[0:128, :])
            ft_tiles = [ft0]

            # the rest of the stream is emitted upfront on the SWDGE ring
            # (one pool buf per chunk -> no WAR deps, ring never stalls)
            xtb = small.tile([128, KC * B], BF16)
            nc.gpsimd.dma_start(xtb[:], inputs_t_d[:])
            for c in range(1, NCH):
                ft_tiles.append(emit_ft_dma(c))

            # targets second on the HWDGE ring (needed only by ~+25us)
            tg = small.tile([B, NSH], F32)
            nc.sync.dma_start(tg[:], targets_d[:])

            # HAM pre-warm: throwaway matmuls while the first feature chunk
            # streams in, so the PE clock gate is at 8/8 (2.4GHz) when the
            # real matmuls start.
            dwarm = psw.tile([B, CW], F32)
            for _ in range(24):
                nc.tensor.matmul(dwarm[:], wsrc[:, 0:B], wsrc[:],
                                 start=True, stop=True)

            # x stats from the bf16 stationary itself (no natural-layout
            # inputs load needed): sqx = xtb^2 on ACT, then a partition-
            # reducing matmul chain (sqx chunk stationary, ones moving)
            # gives ss directly in [B, 1] orientation.
            sqx = small.tile([128, KC * B], BF16)
            nc.scalar.activation(
                sqx[:], xtb[:], mybir.ActivationFunctionType.Square)
            ss_ps = psw.tile([B, 1], F32)
            for k in range(KC):
                nc.tensor.matmul(
                    ss_ps[:], sqx[:, k * B:(k + 1) * B], ones_bf[:],
                    start=(k == 0), stop=(k == KC - 1),
                )
            ss = small.tile([B, 1], F32)
            nc.vector.tensor_copy(ss[:], ss_ps[:])
            # inv = 1/(TEMP*||x_b||): sqrt(ss*TEMP^2) then reciprocal
            srt = small.tile([B, 1], F32)
            nc.scalar.activation(
                srt[:], ss[:], mybir.ActivationFunctionType.Sqrt,
                scale=float(TEMP) * float(TEMP),
            )
            inv = small.tile([B, 1], F32)
            nc.vector.reciprocal(inv[:], srt[:])

            # DVE cast for the HWDGE chunk 0, in halves (emitted after the
            # inv chain so the reciprocal isn't FIFO-blocked behind it)
            halfc = KC * CW // 2
            nc.vector.tensor_copy(ft0[:, 0:halfc], stg0[:, 0:halfc])
            nc.vector.tensor_copy(ft0[:, halfc:], stg0[:, halfc:])

            # targets: u = sum exp(t - 1); etp = exp(t - 1) * inv
            et = small.tile([B, NSH], F32)
            u = small.tile([B, 1], F32)
            nc.scalar.activation(
                et[:], tg[:], mybir.ActivationFunctionType.Exp,
                bias=bias_m1[:], accum_out=u[:],
            )
            etp = small.tile([B, NSH], F32)
            nc.vector.tensor_scalar_mul(etp[:], et[:], inv[:])

            # ---- main stream: per n-chunk, 16 matmuls + epilogue
            s_parts = small.tile([B, NCH], F32)
            p_parts = small.tile([B, NCH], F32)

            sbout = small.tile([B, 3], F32)
            nc.vector.tensor_copy(sbout[:, 1:2], u[:])
            s06 = small.tile([B, 1], F32)
            p06 = small.tile([B, 1], F32)

            for c in range(NCH):
                ft = ft_tiles[c]
                ps = psp.tile([B, CW], F32)
                for k in range(KC):
                    nc.tensor.matmul(
                        ps[:], xtb[:, k * B:(k + 1) * B],
                        ft[:, k * CW:(k + 1) * CW],
                        start=(k == 0), stop=(k == KC - 1),
                    )

                # p_part = sum_n etp * l_raw (DVE mul + reduce; emitted
                # first so it runs concurrently with the ACT exp below)
                pm = epi.tile([B, CW], F32)
                nc.vector.tensor_mul(pm[:], etp[:, c * CW:(c + 1) * CW], ps[:])
                nc.vector.reduce_sum(
                    p_parts[:, c:c + 1], pm[:], axis=mybir.AxisListType.X)
                # s_part = sum_n exp(l_raw*inv - SHIFT)   (fused on ACT)
                el = epi.tile([B, CW], F32)
                nc.scalar.activation(
                    el[:], ps[:], mybir.ActivationFunctionType.Exp,
                    bias=bias_shift[:], scale=inv[:],
                    accum_out=s_parts[:, c:c + 1],
                )
                if c == NCH - 2:
                    # pre-reduce chunks 0..6 while the last chunk streams,
                    # so the final reduction is just two scalar adds
                    nc.vector.reduce_sum(
                        s06[:], s_parts[:, 0:NCH - 1], axis=mybir.AxisListType.X)
                    nc.vector.reduce_sum(
                        p06[:], p_parts[:, 0:NCH - 1], axis=mybir.AxisListType.X)

            # ---- final per-core reduction and output
            nc.vector.tensor_add(
                sbout[:, 0:1], s06[:], s_parts[:, NCH - 1:NCH])
            nc.vector.tensor_add(
                sbout[:, 2:3], p06[:], p_parts[:, NCH - 1:NCH])
            nc.sync.dma_start(out_d[:], sbout[:])

    nc.compile()
    return nc


_NC_CACHE = None


def _stage(inputs):
    """Host-side sharding + layout staging (pure permutations, f32 kept)."""
    x = np.asarray(inputs["inputs"], dtype=np.float32)
    t = np.asarray(inputs["targets"], dtype=np.float32)
    f = np.asarray(inputs["features"], dtype=np.float32)

    # transposed stationary: xt[p, k*B+b] = x[b, k*128+p]
    xt = np.ascontiguousarray(
        x.reshape(B, KC, 128).transpose(2, 1, 0)).reshape(128, KC * B)

    in_maps = []
    for c in range(NUM_CORES):
        fsh = f[c * NSH:(c + 1) * NSH, :]  # [n, d]
        # staged[c, p, k, j] = fsh[c*256+j, k*128+p]
        fstg = np.ascontiguousarray(
            fsh.reshape(NCH, CW, KC, 128).transpose(0, 3, 2, 1)
        ).reshape(NCH * 128, KC * CW)
        # last chunk re-staged half-major: [h*128+p, k'*256+j]
        flast = np.ascontiguousarray(
            fstg[(NCH - 1) * 128:, :].reshape(128, 2, KC * CW // 2)
            .transpose(1, 0, 2)).reshape(2 * 128, KC * CW // 2)
        in_maps.append({
            "inputs_t": xt,
            "targets": np.ascontiguousarray(t[:, c * NSH:(c + 1) * NSH]),
            "features": np.ascontiguousarray(fstg[:(NCH - 1) * 128, :]),
            "features_last": flast,
        })
    return in_maps


def _run(inputs, trace=False, **spmd_kwargs):
    global _NC_CACHE
    from concourse.bass_utils import run_bass_kernel_spmd

    if _NC_CACHE is None:
        _NC_CACHE = build_nc(debug=False)
    nc = _NC_CACHE

    in_maps = _stage(inputs)
    res = run_bass_kernel_spmd(
        nc, in_maps, core_ids=list(range(NUM_CORES)), trace=trace, **spmd_kwargs)
    outs = np.stack([r["out"] for r in res.results])  # [8, B, 3]

    outs64 = outs.astype(np.float64)
    s = outs64[:, :, 0].sum(0)
    u = outs64[:, :, 1].sum(0)
    p = outs64[:, :, 2].sum(0)
    lse = SHIFT + np.log(s)
    loss = np.mean(lse - p / u)
    return np.float32(loss), res


def kernel(**inputs: np.ndarray) -> np.ndarray:
    loss, _ = _run(inputs)
    return np.asarray(loss, dtype=np.float32)


# revision 43
# speedup vs baseline: 1.0453x; 1.0453x over previous
"""Trainium2 Bass kernel for nn_CamMemory (soft cross-entropy vs. memory bank).

Computes: x = normalize(inputs); logits = x @ features.T / TEMP;
loss = mean_b( lse(logits_b) - dot(softmax(targets_b), logits_b) )

Sharding: features/targets split row-wise (N dim) across 8 cores; inputs
replicated.  Each core returns partial stats (s, u, p) per batch row:
  s = sum_n exp(logits - SHIFT)      (partial sum-exp, fixed shift; |logits|<=21)
  u = sum_n exp(targets - 1)         (partial softmax denominator; targets in [0,1))
  p = sum_n exp(targets - 1)*logits  (partial weighted logit sum)
Host combines: loss = mean_b( SHIFT + log(sum s) - (sum p)/(sum u) ).

Layout strategy: the host stages each core's feature shard PRE-TRANSPOSED
and tiled (d on partitions) so the kernel needs no on-chip transposes at
all -- DRAM layout per chunk c of 256 bank rows is [128(d%128), 16(d/128),
256(n)], one contiguous 2MB block.  Inputs are likewise staged twice:
natural [64, 2048] (for the norm stats) and transposed [128, 16*64] (the
matmul stationary).  The kernel is then a pure stream:
  - SWDGE cast-DMA f32 DRAM -> bf16 SBUF, 2MB per n-chunk (8 chunks, all
    emitted upfront on the single SWDGE ring with one pool buf each so the
    ring never stalls; the last chunk is split into two contiguous-staged
    k-halves so the final matmul dependency lands 1MB early).
  - Per chunk: 16 accumulating bf16 matmuls (xT_k stationary, 256-col
    moving) into a PSUM tile [64, 256] = raw logits (no 1/TEMP, no norm).
  - ||x||^2 comes from the bf16 stationary itself (ACT square + a
    partition-reducing matmul chain against a ones column), so no
    natural-layout inputs load is needed at all.
  - Epilogue folds normalization in: exp(l_raw*inv - SHIFT) on ACT with
    per-partition scale inv = 1/(TEMP*||x_b||) and accum_out -> s.
    p uses inv-prescaled exp-targets: DVE mul + row-sum.
The DMA (16.8MB feature read at ~430GB/s) is the critical path; PE/ACT/
DVE all chase it with large headroom.
"""

import numpy as np

import concourse.bacc as bacc
import concourse.mybir as mybir
import concourse.tile as tile

B = 64
D = 2048
N = 16384
NUM_CORES = 8
NSH = N // NUM_CORES  # 2048 rows of features per core
TEMP = 0.05
SHIFT = 21.0  # |logits| <= (1/TEMP)*|x.f| <= 20*(1+eps) since both unit-norm

KC = D // 128     # 16 contraction chunks (d on partitions)
NCH = NSH // 256  # 8 n-chunks of 256 bank rows
CW = 256          # chunk width (bank rows per chunk)

F32 = mybir.dt.float32
BF16 = mybir.dt.bfloat16


def build_nc(debug=False):
    """Build the single-core Bass program (SPMD: same program, 8 shards)."""
    nc = bacc.Bacc("TRN2", target_bir_lowering=False, debug=debug)

    # inputs_t / features are host-staged in transposed+tiled layout, see _run.
    inputs_t_d = nc.dram_tensor("inputs_t", [128, KC * B], F32, kind="ExternalInput")
    targets_d = nc.dram_tensor("targets", [B, NSH], F32, kind="ExternalInput")
    features_d = nc.dram_tensor("features", [(NCH - 1) * 128, KC * CW], F32,
                                kind="ExternalInput")
    # last chunk staged half-major so each k-half is a fully contiguous
    # [128 x 8KB] region (strided or small sub-chunk reads drain slowly)
    features_last_d = nc.dram_tensor("features_last", [2 * 128, KC * CW // 2],
                                     F32, kind="ExternalInput")
    out_d = nc.dram_tensor("out", [B, 3], F32, kind="ExternalOutput")

    with tile.TileContext(nc) as tc:
        with (
            tc.tile_pool(name="small", bufs=1) as small,
            tc.tile_pool(name="ft", bufs=8) as ftp,
            tc.tile_pool(name="epi", bufs=4) as epi,
            tc.tile_pool(name="psw", bufs=1, space="PSUM") as psw,
            tc.tile_pool(name="psum", bufs=4, space="PSUM") as psp,
        ):
            # constants
            bias_m1 = small.tile([B, 1], F32)
            nc.vector.memset(bias_m1[:], -1.0)
            bias_shift = small.tile([B, 1], F32)
            nc.vector.memset(bias_shift[:], -float(SHIFT))
            wsrc = small.tile([128, CW], BF16)
            nc.vector.memset(wsrc[:], 0.0)
            ones_bf = small.tile([128, 1], BF16)
            nc.vector.memset(ones_bf[:], 1.0)

            def emit_ft_dma(c):
                ft = ftp.tile([128, KC * CW], BF16, name="ftc")
                if c < NCH - 1:
                    nc.gpsimd.dma_start(ft[:], features_d[c * 128:(c + 1) * 128, :])
                else:
                    # last chunk split along k into halves so the final
                    # matmul dependency (k=8..15) lands 1MB earlier; both
                    # DMA sides stay per-partition contiguous (smaller
                    # sub-chunks trigger an SDMA-15 slowdown)
                    q = KC * CW // 2
                    for h in range(2):
                        nc.gpsimd.dma_start(
                            ft[:, h * q:(h + 1) * q],
                            features_last_d[h * 128:(h + 1) * 128, :])
                return ft

            # the whole feature stream is emitted upfront on the SWDGE ring
            # (one pool buf per chunk -> no WAR deps, ring never stalls)
            ft_tiles = [emit_ft_dma(0)]
            xtb = small.tile([128, KC * B], BF16)
            nc.gpsimd.dma_start(xtb[:], inputs_t_d[:])
            for c in range(1, NCH):
                ft_tiles.append(emit_ft_dma(c))

            # targets on HWDGE at t0 (least-bad slot for its 64-partition
            # descriptor spray: the SWDGE stream hasn't ramped yet)
            tg = small.tile([B, NSH], F32)
            nc.sync.dma_start(tg[:], targets_d[:])

            # HAM pre-warm: throwaway matmuls while the first feature chunk
            # streams in, so the PE clock gate is at 8/8 (2.4GHz) when the
            # real matmuls start.
            dwarm = psw.tile([B, CW], F32)
            for _ in range(24):
                nc.tensor.matmul(dwarm[:], wsrc[:, 0:B], wsrc[:],
                                 start=True, stop=True)

            # x stats from the bf16 stationary itself (no natural-layout
            # inputs load needed): sqx = xtb^2 on ACT, then a partition-
            # reducing matmul chain (sqx chunk stationary, ones moving)
            # gives ss directly in [B, 1] orientation.
            sqx = small.tile([128, KC * B], BF16)
            nc.scalar.activation(
                sqx[:], xtb[:], mybir.ActivationFunctionType.Square)
            ss_ps = psw.tile([B, 1], F32)
            for k in range(KC):
                nc.tensor.matmul(
                    ss_ps[:], sqx[:, k * B:(k + 1) * B], ones_bf[:],
                    start=(k == 0), stop=(k == KC - 1),
                )
            ss = small.tile([B, 1], F32)
            nc.vector.tensor_copy(ss[:], ss_ps[:])
            # inv = 1/(TEMP*||x_b||): sqrt(ss*TEMP^2) then reciprocal
            srt = small.tile([B, 1], F32)
            nc.scalar.activation(
                srt[:], ss[:], mybir.ActivationFunctionType.Sqrt,
                scale=float(TEMP) * float(TEMP),
            )
            inv = small.tile([B, 1], F32)
            nc.vector.reciprocal(inv[:], srt[:])

            # targets: u = sum exp(t - 1); etp = exp(t - 1) * inv
            et = small.tile([B, NSH], F32)
            u = small.tile([B, 1], F32)
            nc.scalar.activation(
                et[:], tg[:], mybir.ActivationFunctionType.Exp,
                bias=bias_m1[:], accum_out=u[:],
            )
            etp = small.tile([B, NSH], F32)
            nc.vector.tensor_scalar_mul(etp[:], et[:], inv[:])

            # ---- main stream: per n-chunk, 16 matmuls + epilogue
            s_parts = small.tile([B, NCH], F32)
            p_parts = small.tile([B, NCH], F32)

            sbout = small.tile([B, 3], F32)
            nc.vector.tensor_copy(sbout[:, 1:2], u[:])
            s06 = small.tile([B, 1], F32)
            p06 = small.tile([B, 1], F32)

            for c in range(NCH):
                ft = ft_tiles[c]
                ps = psp.tile([B, CW], F32)
                for k in range(KC):
                    nc.tensor.matmul(
                        ps[:], xtb[:, k * B:(k + 1) * B],
                        ft[:, k * CW:(k + 1) * CW],
                        start=(k == 0), stop=(k == KC - 1),
                    )

                # p_part = sum_n etp * l_raw (DVE mul + reduce; emitted
                # first so it runs concurrently with the ACT exp below)
                pm = epi.tile([B, CW], F32)
                nc.vector.tensor_mul(pm[:], etp[:, c * CW:(c + 1) * CW], ps[:])
                nc.vector.reduce_sum(
                    p_parts[:, c:c + 1], pm[:], axis=mybir.AxisListType.X)
                # s_part = sum_n exp(l_raw*inv - SHIFT)   (fused on ACT)
                el = epi.tile([B, CW], F32)
                nc.scalar.activation(
                    el[:], ps[:], mybir.ActivationFunctionType.Exp,
                    bias=bias_shift[:], scale=inv[:],
                    accum_out=s_parts[:, c:c + 1],
                )
                if c == NCH - 2:
                    # pre-reduce chunks 0..6 while the last chunk streams,
                    # so the final reduction is just two scalar adds
                    nc.vector.reduce_sum(
                        s06[:], s_parts[:, 0:NCH - 1], axis=mybir.AxisListType.X)
                    nc.vector.reduce_sum(
                        p06[:], p_parts[:, 0:NCH - 1], axis=mybir.AxisListType.X)

            # ---- final per-core reduction and output
            nc.vector.tensor_add(
                sbout[:, 0:1], s06[:], s_parts[:, NCH - 1:NCH])
            nc.vector.tensor_add(
                sbout[:, 2:3], p06[:], p_parts[:, NCH - 1:NCH])
            nc.sync.dma_start(out_d[:], sbout[:])

    nc.compile()
    return nc


_NC_CACHE = None


def _stage(inputs):
    """Host-side sharding + layout staging (pure permutations, f32 kept)."""
    x = np.asarray(inputs["inputs"], dtype=np.float32)
    t = np.asarray(inputs["targets"], dtype=np.float32)
    f = np.asarray(inputs["features"], dtype=np.float32)

    # transposed stationary: xt[p, k*B+b] = x[b, k*128+p]
    xt = np.ascontiguousarray(
        x.reshape(B, KC, 128).transpose(2, 1, 0)).reshape(128, KC * B)

    in_maps = []
    for c in range(NUM_CORES):
        fsh = f[c * NSH:(c + 1) * NSH, :]  # [n, d]
        # staged[c, p, k, j] = fsh[c*256+j, k*128+p]
        fstg = np.ascontiguousarray(
            fsh.reshape(NCH, CW, KC, 128).transpose(0, 3, 2, 1)
        ).reshape(NCH * 128, KC * CW)
        # last chunk re-staged half-major: [h*128+p, k'*256+j]
        flast = np.ascontiguousarray(
            fstg[(NCH - 1) * 128:, :].reshape(128, 2, KC * CW // 2)
            .transpose(1, 0, 2)).reshape(2 * 128, KC * CW // 2)
        in_maps.append({
            "inputs_t": xt,
            "targets": np.ascontiguousarray(t[:, c * NSH:(c + 1) * NSH]),
            "features": np.ascontiguousarray(fstg[:(NCH - 1) * 128, :]),
            "features_last": flast,
        })
    return in_maps


def _run(inputs, trace=False, **spmd_kwargs):
    global _NC_CACHE
    from concourse.bass_utils import run_bass_kernel_spmd

    if _NC_CACHE is None:
        _NC_CACHE = build_nc(debug=False)
    nc = _NC_CACHE

    in_maps = _stage(inputs)
    res = run_bass_kernel_spmd(
        nc, in_maps, core_ids=list(range(NUM_CORES)), trace=trace, **spmd_kwargs)
    outs = np.stack([r["out"] for r in res.results])  # [8, B, 3]

    outs64 = outs.astype(np.float64)
    s = outs64[:, :, 0].sum(0)
    u = outs64[:, :, 1].sum(0)
    p = outs64[:, :, 2].sum(0)
    lse = SHIFT + np.log(s)
    loss = np.mean(lse - p / u)
    return np.float32(loss), res


def kernel(**inputs: np.ndarray) -> np.ndarray:
    loss, _ = _run(inputs)
    return np.asarray(loss, dtype=np.float32)
